# revision 30
# baseline (speedup 1.0000x reference)
"""Trainium2 Bass kernel (raw Bass, explicit semaphores) for a BiDAF-style
attention-flow layer — bf16 restructure, software-pipelined.

Math (per batch b):
    S[t,j] = c.w_c + q.w_q + (c*q).w_cq, masked by (t<con_len)&(j<qu_len)
    c2q    = softmax_j(S) @ Q
    value  = softmax_t(max_j S);  q2c = sum_t value[t] * C[t]
    G      = [C, c2q, C*c2q, C*q2c] * t_valid

Device design (vs the fp32 t-major baseline):
  - All matmuls in bf16 (1 cyc/row vs fp32's 4); PSUM accumulates fp32.
  - S is computed TRANSPOSED (S'[j,t]) so q_proj[j] + jmask[j] folds into
    the per-partition bias of the exp activation, and exp(S') IS the
    lhsT P^T needed by the c2q matmul — no PT transposes / pts copies.
  - Value path: P^T is transposed back per chunk on PE (bf16) and DVE
    reduce_max/reduce_sum over free-j give pm8 and the softmax sums.
  - NO narrow (N=1) bf16 matmuls: they corrupt PSUM/crash on real HW at
    pipeline rate (found by bisection; the fp32 baseline was immune).
    * c_proj+tmask (x1) and q_proj+jmask (exp bias) are host-precomputed
      mask-style aux columns (tiny linear input projections).
    * q2c^T uses N=8-wide accumulating matmuls against an 8x8
      identity-masked e8 (e8m), summed with one DVE reduce.
    * the value-sum total is an fp32 narrow matmul (fp32 is immune).
  - Pipelining: loads on SP (triple-buffered inputs, never blocked);
    stores issued from the Pool queue (SWDGE) after its G3 op; the ACT
    G1/q2c tail ops and PE F/G stages are shifted one iteration so
    exp(b) never queues behind batch b-1's value-chain tail.
  - G0 (= masked context, a verbatim input copy) is assembled on host;
    device emits only [c2q, C*c2q, C*q2c] as bf16 (host upcasts):
    store DMA 16 MB -> 6 MB per core. Output is stored per-partition
    contiguous (128 descriptors/batch); host un-permutes.
Sharding: data-parallel over batch B=32 across 8 NeuronCores (4 each).
"""

import os
import sys
import functools

BISECT = int(os.environ.get("KBISECT", "0"))   # 1 = drop value path (debug)

for _p in ("/opt/trn_rl_repo",):
    if _p not in sys.path:
        sys.path.insert(0, _p)

import numpy as np
import ml_dtypes
import concourse.bass as bass
from concourse import mybir

T, J, B, D = 1024, 128, 32, 256
NCORES = 8
BL = B // NCORES
NT = T // 128
NEG = -1.0e30

F32 = mybir.dt.float32
BF16 = mybir.dt.bfloat16
AX = mybir.AxisListType.X
EXP = mybir.ActivationFunctionType.Exp
ADD = mybir.AluOpType.add

DMA_SEMS = {"wsb"} | {f"{k}{i}" for k in "cxqtmg" for i in range(3)}


class Em:
    """Per-engine emitter: dry pass counts sem values, real pass emits."""

    def __init__(self, dry, ctr, ev, eng=None, sems=None):
        self.dry = dry
        self.ctr = ctr
        self.ev = ev
        self.eng = eng
        self.sems = sems
        self.waited = {}

    def do(self, fn, sem=None, tag=None):
        inst = None if self.dry else fn()
        if sem is not None:
            step = 16 if sem in DMA_SEMS else 1
            if inst is not None:
                inst.then_inc(self.sems[sem], step)
            self.ctr[sem] += step
            if tag is not None:
                self.ev[tag] = (sem, self.ctr[sem])
        return inst

    def mark(self, tag, sem):
        self.ev[tag] = (sem, self.ctr[sem])

    def w(self, tag):
        if self.dry:
            return
        if tag not in self.ev:
            return
        sem, val = self.ev[tag]
        if val <= 0:
            return
        if self.waited.get(sem, 0) >= val:
            return
        self.eng.wait_ge(self.sems[sem], val)
        self.waited[sem] = val


def build():
    nc = bass.Bass("TRN2", target_bir_lowering=False, debug=False)

    cna_d = nc.dram_tensor("cnab", (BL, 128, NT * D), BF16, kind="ExternalInput").ap()
    ctxT_d = nc.dram_tensor("ctxTb", (BL, 128, 2 * NT * 128), BF16, kind="ExternalInput").ap()
    qn_d = nc.dram_tensor("qnb", (BL, 128, D), BF16, kind="ExternalInput").ap()
    qT_d = nc.dram_tensor("qTb", (BL, 128, 2 * J), BF16, kind="ExternalInput").ap()
    aux_d = nc.dram_tensor("auxf", (BL, 128, 20), F32, kind="ExternalInput").ap()
    wsb_d = nc.dram_tensor("wsb", (128, 68), BF16, kind="ExternalInput").ap()
    # [b, p, c, 3*256]: G row t = c*128 + p (host un-permutes)
    out_d = nc.dram_tensor("out", (BL, 128, NT * 3 * D), BF16, kind="ExternalOutput").ap()

    A = lambda name, shape, dt=BF16: nc.alloc_sbuf_tensor(name, list(shape), dt).ap()

    identb = A("identb", (128, 128))
    ones_row = A("ones_row", (1, 128))
    onesF = A("onesF", (128, 1), F32)
    wsb = A("wsb_s", (128, 68))
    i8v = wsb[:, 4:68].rearrange("p (h k) -> p h k", k=8)
    cna = [A(f"cna{i}", (128, NT, D)) for i in range(3)]
    ctxT = [A(f"ctxT{i}", (128, 2, NT, 128)) for i in range(3)]
    qn = [A(f"qn{i}", (128, D)) for i in range(3)]
    qT = [A(f"qT{i}", (128, 2 * J)) for i in range(3)]
    aux = [A(f"aux{i}", (128, 20), F32) for i in range(3)]
    qwT = [A(f"qwT{i}", (128, 2 * J)) for i in range(2)]
    PT = [A(f"PT{i}", (128, NT, 128)) for i in range(2)]
    pm8 = [A(f"pm8_{i}", (128, NT), F32) for i in range(2)]
    ss8 = [A(f"ss8_{i}", (128, NT), F32) for i in range(2)]
    rcp8 = [A(f"rcp8_{i}", (128, NT), F32) for i in range(2)]
    rs018 = [A(f"rs018_{i}", (128, NT), F32) for i in range(2)]
    ex8 = [A(f"ex8_{i}", (128, NT), F32) for i in range(2)]
    e8 = [A(f"e8_{i}", (128, NT)) for i in range(2)]
    e8m = [A(f"e8m_{i}", (128, NT, NT)) for i in range(2)]
    ered = [A(f"ered_{i}", (128, 1), F32) for i in range(2)]
    rtot = [A(f"rtot_{i}", (1, 1), F32) for i in range(2)]
    q2cTf = [A(f"q2cTf{i}", (128, 2), F32) for i in range(2)]
    q2cTs = [A(f"q2cTs{i}", (128, 2)) for i in range(2)]
    q2c_sb = [A(f"q2c_sb{i}", (1, D)) for i in range(2)]
    q2cb = [A(f"q2cb{i}", (128, D)) for i in range(2)]
    gbig = [A(f"gbig{i}", (128, NT, 3, D)) for i in range(3)]

    P = lambda name, shape, dt=F32: nc.alloc_psum_tensor(name, list(shape), dt).ap()
    sp = [P(f"sp{i}", (128, 512)) for i in range(2)]        # S' lo/hi (4 chunks each)
    c2q_ps = P("c2q_ps", (128, NT * D))                      # 4 banks
    pback = P("pback", (128, NT * 128), BF16)                # 1 bank, bf16
    auxp = P("auxp", (128, 512))                             # 1 bank
    q2cT8 = [auxp[:, 0:8], auxp[:, 8:16]]                    # (128, 8) each
    q2cT8v = auxp[:, 0:16].rearrange("p (a k) -> p a k", k=8)
    tot_ps = auxp[0:1, 24:25]
    q2c_rowb = auxp[0:1, 32:160].bitcast(BF16)               # (1, 256) bf16
    q2cb_ps = auxp[:, 256:512]

    sem_names = list(DMA_SEMS) + ["pe", "act", "dve", "pool"]
    sems = {n: nc.alloc_semaphore(f"sem_{n}") for n in sem_names}

    pbv = pback.rearrange("p (c j) -> p c j", j=128)

    # ------------------------------------------------------------------ streams
    def stream_sync(X):
        X.do(lambda: nc.sync.dma_start(out=wsb, in_=wsb_d), "wsb", "wsb")
        for b in range(BL):
            b3 = b % 3
            # loads for b (buffer-free waits are on batch b-3 consumers)
            X.w(f"q2cTmm_{b-3}"); X.w(f"G2_{b-3}"); X.w(f"G3_{b-3}")
            X.do(lambda b=b, b3=b3: nc.sync.dma_start(
                out=cna[b3].rearrange("p c d -> p (c d)"), in_=cna_d[b]),
                f"c{b3}", f"c_{b}")
            X.w(f"sh_{b-3}")
            X.do(lambda b=b, b3=b3: nc.sync.dma_start(
                out=ctxT[b3].rearrange("p a c t -> p (a c t)"), in_=ctxT_d[b]),
                f"x{b3}", f"x_{b}")
            X.w(f"c2qh_{b-3}")
            X.do(lambda b=b, b3=b3: nc.sync.dma_start(out=qn[b3], in_=qn_d[b]),
                 f"q{b3}", f"q_{b}")
            X.w(f"qwT_{b-3}")
            X.do(lambda b=b, b3=b3: nc.sync.dma_start(out=qT[b3], in_=qT_d[b]),
                 f"t{b3}", f"t_{b}")
            X.w(f"rs018_{b-3}"); X.w(f"ex8_{b-3}"); X.w(f"exph_{b-3}")
            X.do(lambda b=b, b3=b3: nc.sync.dma_start(out=aux[b3], in_=aux_d[b]),
                 f"m{b3}", f"m_{b}")

    def stream_pool(X):
        NE = mybir.AluOpType.not_equal
        X.do(lambda: nc.gpsimd.memset(identb, 0.0), "pool", "identms")
        if not X.dry:
            X.eng.wait_ge(sems["pool"], X.ev["identms"][1])
        X.do(lambda: nc.gpsimd.affine_select(
            out=identb, in_=identb, compare_op=NE, fill=1.0, base=0,
            pattern=[[-1, 128]], channel_multiplier=1), "pool")
        X.do(lambda: nc.gpsimd.memset(ones_row, 1.0), "pool")
        X.do(lambda: nc.gpsimd.memset(onesF, 1.0), "pool", "consts")
        for b in range(BL):
            b3 = b % 3
            be = b % 2
            if BISECT != 1:
                # G3 = cna * q2cb
                X.w(f"q2cbc_{b}")
                X.w(f"c_{b}")
                X.w(f"gfree_{b-3}")
                X.do(lambda be=be, b3=b3: nc.gpsimd.tensor_mul(
                    gbig[b3][:, :, 2, :], cna[b3],
                    q2cb[be].unsqueeze(1).broadcast_to((128, NT, D))),
                    "pool", f"G3_{b}")
            else:
                X.w(f"c_{b}")
                X.w(f"gfree_{b-3}")
                X.do(lambda b3=b3: nc.gpsimd.tensor_copy(
                    gbig[b3][:, :, 2, :], cna[b3]), "pool", f"G3_{b}")
            # store from the pool queue (SWDGE): SP loads never block
            X.w(f"G2_{b}"); X.w(f"G1a_{b}"); X.w(f"G1d_{b}"); X.w(f"G3_{b}")
            X.do(lambda b=b, b3=b3: nc.gpsimd.dma_start(
                out=out_d[b],
                in_=gbig[b3].rearrange("p c three d -> p (c three d)")),
                f"g{b3}", f"gfree_{b}")

    def stream_pe(X):
        X.w("consts")
        X.w("wsb")
        for b in range(BL):
            b3 = b % 3
            be = b % 2
            # A/B: S' halves (4 chunks each)
            X.w(f"x_{b}")
            X.w(f"qwT_{b}")
            for half in range(2):
                X.w(f"exp{'lh'[half]}_{b-1}")  # sp[half] free
                for hh in range(4):
                    h = half * 4 + hh
                    X.do(lambda be=be, b3=b3, half=half, h=h, hh=hh: nc.tensor.matmul(
                        sp[half][:, hh * 128:(hh + 1) * 128],
                        qwT[be][:, 0:128], ctxT[b3][:, 0, h, :],
                        start=True, stop=False))
                    X.do(lambda be=be, b3=b3, half=half, h=h, hh=hh: nc.tensor.matmul(
                        sp[half][:, hh * 128:(hh + 1) * 128],
                        qwT[be][:, 128:256], ctxT[b3][:, 1, h, :],
                        start=False, stop=True),
                        "pe", f"s{'lh'[half]}_{b}" if hh == 3 else None)
            if BISECT != 1 and b >= 1:
                pe_fg(X, b - 1)
            # C/D: c2q + pback per half
            X.w(f"q_{b}")
            for half in range(2):
                X.w(f"exp{'lh'[half]}_{b}")  # PT half ready
                if half == 0:
                    X.w(f"G1d_{b-1}")  # c2q_ps lo free (DVE reader)
                    X.w(f"maxred_{b-1}"); X.w(f"sumred_{b-1}")  # pback free
                else:
                    X.w(f"G1a_{b-1}")  # c2q_ps hi free (ACT reader)
                for hh in range(4):
                    h = half * 4 + hh
                    X.do(lambda be=be, b3=b3, h=h: nc.tensor.matmul(
                        c2q_ps[:, h * D:(h + 1) * D], PT[be][:, h, :], qn[b3],
                        start=True, stop=True),
                        "pe", f"c2q{'lh'[half]}_{b}" if hh == 3 else None)
                    X.do(lambda be=be, h=h: nc.tensor.transpose(
                        pback[:, h * 128:(h + 1) * 128], PT[be][:, h, :], identb),
                        "pe", f"pb{'lh'[half]}_{b}" if hh == 3 else None)
            X.mark(f"ptfree_{b}", "pe")
            if BISECT == 1:
                continue
            # E: q2c^T via N=8 identity-masked accumulating matmuls
            X.w(f"e8m_{b}")
            X.w(f"q2cTf_{b-1}")  # q2cT8 psum free (DVE reader)
            for half in range(2):
                for h in range(NT):
                    X.do(lambda be=be, b3=b3, h=h, half=half: nc.tensor.matmul(
                        q2cT8[half], cna[b3][:, h, 128 * half:128 * (half + 1)],
                        e8m[be][:, h, :],
                        start=(h == 0), stop=(h == NT - 1)),
                        "pe" if (h == NT - 1 and half == 1) else None,
                        f"q2cTmm_{b}" if (h == NT - 1 and half == 1) else None)
            # value total (fp32 narrow matmul: fp32 is HW-safe)
            X.w(f"ered_{b}")
            X.w(f"rtot_{b-1}")  # tot_ps free
            X.do(lambda be=be: nc.tensor.matmul(
                tot_ps, ered[be], onesF, start=True, stop=True),
                "pe", f"totmm_{b}")
        if BISECT != 1:
            pe_fg(X, BL - 1)

    def pe_fg(X, b):
        be = b % 2
        # F: q2c row transposes
        X.w(f"q2cTs_{b}")
        X.w(f"q2csb_{b-1}")  # q2c_rowb free (DVE reader)
        X.do(lambda be=be: nc.tensor.transpose(
            q2c_rowb[:, 0:128], q2cTs[be][:, 0:1], identb))
        X.do(lambda be=be: nc.tensor.transpose(
            q2c_rowb[:, 128:256], q2cTs[be][:, 1:2], identb),
            "pe", f"q2cTT_{b}")
        # G: q2cb rank-1 broadcast (K=1, N=256 — wide write)
        X.w(f"q2csb_{b}")
        X.w(f"q2cbc_{b-1}")  # q2cb_ps free (ACT reader)
        X.do(lambda be=be: nc.tensor.matmul(
            q2cb_ps, ones_row, q2c_sb[be], start=True, stop=True),
            "pe", f"q2cbmm_{b}")

    def stream_act(X):
        for b in range(BL):
            b3 = b % 3
            be = b % 2
            # qwT for this batch
            X.w(f"t_{b}"); X.w(f"m_{b}")
            X.w(f"sh_{b-2}")  # qwT[be] free
            X.do(lambda be=be, b3=b3: nc.scalar.mul(
                qwT[be][:, 0:128], qT[b3][:, 0:128], aux[b3][:, 17:18]))
            X.do(lambda be=be, b3=b3: nc.scalar.mul(
                qwT[be][:, 128:256], qT[b3][:, 128:256], aux[b3][:, 18:19]),
                "act", f"qwT_{b}")
            # value path exp (x1 = c_proj + tmask is host-precomputed)
            if BISECT != 1:
                X.w(f"e8_{b-2}")  # ex8[be] free (DVE reader)
                X.do(lambda be=be, b3=b3: nc.scalar.activation(
                    ex8[be], aux[b3][:, 8:16], EXP), "act", f"ex8_{b}")
            # exp halves (bias = q_proj + jmask, host-precomputed)
            for half in range(2):
                X.w(f"s{'lh'[half]}_{b}")
                if half == 0:
                    X.w(f"ptfree_{b-2}")  # PT[be] free
                X.do(lambda be=be, b3=b3, half=half: nc.scalar.activation(
                    PT[be][:, half * 4:half * 4 + 4, :], sp[half], EXP,
                    bias=aux[b3][:, 16:17]), "act", f"exp{'lh'[half]}_{b}")
            # shifted tail for batch b-1
            if b >= 1:
                act_tail(X, b - 1)
        act_tail(X, BL - 1)

    def act_tail(X, b):
        b3 = b % 3
        be = b % 2
        # G1 chunks 4-7 (per-chunk: scale by rs01 column)
        X.w(f"c2qh_{b}")
        X.w(f"rs018_{b}")
        X.w(f"gfree_{b-3}")
        for h in range(4, 8):
            X.do(lambda be=be, b3=b3, h=h: nc.scalar.mul(
                gbig[b3][:, h, 0, :], c2q_ps[:, h * D:(h + 1) * D],
                rs018[be][:, h:h + 1]),
                "act", f"G1a_{b}" if h == 7 else None)
        if BISECT == 1:
            return
        # q2cTs: bf16 copy of the DVE-reduced q2c^T halves
        X.w(f"q2cTf_{b}")
        X.w(f"q2cTT_{b-2}")  # q2cTs[be] free (PE reader)
        X.do(lambda be=be: nc.scalar.copy(q2cTs[be], q2cTf[be]),
             "act", f"q2cTs_{b}")
        X.w(f"q2cbmm_{b}")
        X.w(f"G3_{b-2}")  # q2cb[be] free (pool reader)
        X.do(lambda be=be: nc.scalar.copy(q2cb[be], q2cb_ps),
             "act", f"q2cbc_{b}")

    def stream_dve(X):
        for b in range(BL):
            b3 = b % 3
            be = b % 2
            if BISECT != 1 and b >= 1:
                dve_head(X, b - 1)
            X.w(f"pbh_{b}")
            X.do(lambda be=be: nc.vector.reduce_max(pm8[be], pbv, axis=AX),
                 "dve", f"maxred_{b}")
            X.do(lambda be=be: nc.vector.reduce_sum(ss8[be], pbv, axis=AX),
                 "dve", f"sumred_{b}")
            X.w(f"sumred_{b}")
            X.do(lambda be=be: nc.vector.reciprocal(rcp8[be], ss8[be]), "dve",
                 f"rcp_{b}")
            X.w(f"rcp_{b}"); X.w(f"m_{b}")
            X.w(f"G1a_{b-2}")  # rs018[be] free (ACT reader)
            X.do(lambda be=be, b3=b3: nc.vector.tensor_mul(
                rs018[be], rcp8[be], aux[b3][:, 0:8]), "dve", f"rs018_{b}")
            # G1 chunks 0-3 (one op; gates next batch's c2q-lo)
            X.w(f"c2ql_{b}")
            X.w(f"rs018_{b}")
            X.w(f"gfree_{b-3}")
            X.do(lambda be=be, b3=b3: nc.vector.tensor_mul(
                gbig[b3][:, 0:4, 0, :],
                c2q_ps[:, 0:4 * D].rearrange("p (c d) -> p c d", d=D),
                rs018[be][:, 0:4].unsqueeze(2).broadcast_to((128, 4, D))),
                "dve", f"G1d_{b}")
            if BISECT == 1:
                # G2 directly (no value path)
                X.w(f"G1a_{b}")
                X.do(lambda b3=b3: nc.vector.tensor_mul(
                    gbig[b3][:, :, 1, :], cna[b3], gbig[b3][:, :, 0, :]),
                    "dve", f"G2_{b}")
                continue
            X.w(f"ex8_{b}")
            X.w(f"maxred_{b}")
            X.do(lambda be=be: nc.vector.tensor_mul(e8[be], pm8[be], ex8[be]),
                 "dve", f"e8_{b}")
            # e8m[t, h, k] = e8[t, h] * (h == k)
            X.w(f"e8_{b}")
            X.w(f"q2cTmm_{b-2}")  # e8m[be] free (PE reader)
            X.do(lambda be=be: nc.vector.tensor_mul(
                e8m[be], e8[be].unsqueeze(2).broadcast_to((128, NT, NT)),
                i8v), "dve", f"e8m_{b}")
            X.w(f"totmm_{b-2}")  # ered[be] free (PE reader)
            X.do(lambda be=be: nc.vector.reduce_sum(ered[be], e8[be], axis=AX),
                 "dve", f"ered_{b}")
            # q2cTf = sum_h of the masked matmul columns
            X.w(f"q2cTmm_{b}")
            X.w(f"q2cTs_{b-2}")  # q2cTf[be] free (ACT reader)
            X.do(lambda be=be: nc.vector.reduce_sum(
                q2cTf[be], q2cT8v, axis=AX), "dve", f"q2cTf_{b}")
            X.w(f"totmm_{b}")
            X.do(lambda be=be: nc.vector.reciprocal(rtot[be], tot_ps),
                 "dve", f"rtot_{b}")
        if BISECT != 1:
            dve_head(X, BL - 1)

    def dve_head(X, b):
        b3 = b % 3
        be = b % 2
        # q2csb(b): normalized q2c row (runs early in iteration b+1)
        X.w(f"q2cTT_{b}")
        X.w(f"rtot_{b}")
        X.w(f"q2cbmm_{b-2}")  # q2c_sb[be] free (PE reader)
        X.do(lambda be=be: nc.vector.tensor_scalar_mul(
            q2c_sb[be], q2c_rowb, rtot[be]), "dve", f"q2csb_{b}")
        # G2 = cna * G1 (one op)
        X.w(f"G1a_{b}")
        X.w(f"G1d_{b}")
        X.w(f"c_{b}")
        X.do(lambda b3=b3: nc.vector.tensor_mul(
            gbig[b3][:, :, 1, :], cna[b3], gbig[b3][:, :, 0, :]),
            "dve", f"G2_{b}")

    streams = [("sync", stream_sync), ("gpsimd", stream_pool),
               ("tensor", stream_pe), ("scalar", stream_act),
               ("vector", stream_dve)]

    # pass 1: dry run to collect events
    ev = {}
    ctr = {n: 0 for n in sem_names}
    for _, s in streams:
        s(Em(True, ctr, ev, None, None))
    dry_ctr = dict(ctr)

    # pass 2: real emission
    ctr2 = {n: 0 for n in sem_names}
    with nc.Block() as block:

        @block.sync
        def _(eng):
            stream_sync(Em(False, ctr2, ev, eng, sems))

        @block.gpsimd
        def _(eng):
            stream_pool(Em(False, ctr2, ev, eng, sems))

        @block.tensor
        def _(eng):
            stream_pe(Em(False, ctr2, ev, eng, sems))

        @block.scalar
        def _(eng):
            stream_act(Em(False, ctr2, ev, eng, sems))

        @block.vector
        def _(eng):
            stream_dve(Em(False, ctr2, ev, eng, sems))

    assert ctr2 == dry_ctr, (ctr2, dry_ctr)
    return nc


@functools.lru_cache(maxsize=1)
def _build_cached():
    return build()


def _host_prep(context, question, con_lens, qu_lens, att_w):
    bf = ml_dtypes.bfloat16
    ctx = np.asarray(context, dtype=np.float32)      # (T, B, D)
    q = np.asarray(question, dtype=np.float32)       # (J, B, D)
    con = np.asarray(con_lens).astype(np.int64)
    qu = np.asarray(qu_lens).astype(np.int64)
    w = np.asarray(att_w, dtype=np.float32).reshape(3, D)

    t01 = (np.arange(T)[None, :] < con[:, None]).astype(np.float32)   # (B, T)
    ctxz = np.ascontiguousarray(ctx * t01.T[:, :, None])              # zeroed pads
    ctx_bt = ctxz.transpose(1, 0, 2)                                  # (B, T, D)

    cnab = np.ascontiguousarray(
        ctx_bt.reshape(B, NT, 128, D).transpose(0, 2, 1, 3)
        .reshape(B, 128, NT * D).astype(bf))
    ctxTb = np.ascontiguousarray(
        ctx_bt.transpose(0, 2, 1)                      # (B, D, T)
        .reshape(B, 2, 128, NT, 128).transpose(0, 2, 1, 3, 4)
        .reshape(B, 128, 2 * NT * 128).astype(bf))
    qnb = np.ascontiguousarray(q.transpose(1, 0, 2).astype(bf))        # (B, J, D)
    qTb = np.ascontiguousarray(
        q.transpose(1, 2, 0)                            # (B, D, J)
        .reshape(B, 2, 128, J).transpose(0, 2, 1, 3)
        .reshape(B, 128, 2 * J).astype(bf))
    t01t = t01.reshape(B, NT, 128).transpose(0, 2, 1)   # (B, 128, NT)
    # tiny host-side linear projections (narrow device matmuls are unsafe)
    c_proj = np.einsum("tbd,d->bt", ctx, w[0])          # (B, T)
    q_proj = np.einsum("jbd,d->bj", q, w[1])            # (B, J)
    cpt = c_proj.reshape(B, NT, 128).transpose(0, 2, 1)  # (B, 128, NT)
    auxf = np.zeros((B, 128, 20), dtype=np.float32)
    auxf[:, :, 0:8] = t01t
    auxf[:, :, 8:16] = cpt + (1.0 - t01t) * NEG          # x1 = c_proj + tmask
    auxf[:, :, 16] = q_proj + np.where(
        np.arange(J)[None, :] < qu[:, None], 0.0, NEG)   # exp bias
    auxf[:, :, 17] = w[2, 0:128][None, :]
    auxf[:, :, 18] = w[2, 128:256][None, :]
    wsbf = np.zeros((128, 68), dtype=np.float32)
    wsbf[:, 4:68] = np.eye(NT, dtype=np.float32).reshape(1, NT * NT)
    wsb = np.ascontiguousarray(wsbf.astype(bf))
    return cnab, ctxTb, qnb, qTb, auxf, wsb, ctx_bt


def kernel(context, question, con_lens, qu_lens, att_w):
    from concourse.bass_utils import run_bass_kernel_spmd

    cnab, ctxTb, qnb, qTb, auxf, wsb, ctx_bt = _host_prep(
        context, question, con_lens, qu_lens, att_w)
    in_maps = []
    for i in range(NCORES):
        sl = slice(i * BL, (i + 1) * BL)
        in_maps.append({
            "cnab": np.ascontiguousarray(cnab[sl]),
            "ctxTb": np.ascontiguousarray(ctxTb[sl]),
            "qnb": np.ascontiguousarray(qnb[sl]),
            "qTb": np.ascontiguousarray(qTb[sl]),
            "auxf": np.ascontiguousarray(auxf[sl]),
            "wsb": wsb,
        })
    nc = _build_cached()
    res = run_bass_kernel_spmd(nc, in_maps, core_ids=list(range(NCORES)))
    # device layout: [b, p, c, 768] with G row t = c*128 + p
    dev = np.concatenate(
        [np.asarray(res.results[i]["out"]).reshape(BL, 128, NT, 3 * D)
         for i in range(NCORES)], axis=0).astype(np.float32)
    dev = dev.transpose(0, 2, 1, 3).reshape(B, T, 3 * D)
    out = np.empty((B, T, 4 * D), dtype=np.float32)
    out[:, :, 0:D] = ctx_bt          # G0 = masked context (verbatim input)
    out[:, :, D:] = dev              # [c2q, C*c2q, C*q2c]
    return out
